# revision 20
# baseline (speedup 1.0000x reference)
"""Trainium2 Bass kernel for the Approx_OTPE SNN forward pass.

Reference computation (T=32, B=128, IN=OUT=2048, fp32):
    u' = sigmoid(2.0)*u + x[t] @ W ; s = (u' >= 1) ; u = u' - s
    returns stacked spikes [T, B, OUT]

Strategy: data-parallel over batch B across 8 NeuronCores (B_local=16, W
replicated, no collectives).  Per core the matmul Z = x_flat @ W
([512,2048] @ [2048,2048]) is time-independent, so it is computed on the
TensorEngine while the cheap nonlinear scan (decay + threshold + soft
reset) follows behind on the VectorEngine.

Matmul modes:
  - "bf16x3" (default): x and W are split hi/lo into bfloat16 pairs ON THE
    HOST (xh+xl ~= x, wh+wl ~= W to ~2^-17 relative error) and x is also
    pre-transposed on the host.  The device accumulates the three cross
    terms xh*wh + xh*wl + xl*wh in fp32 PSUM — 3 full-rate bf16 passes
    instead of the PE's 4-cycle fp32 mode, with ~5e-6 z-error (far below
    the ~1e-4 level where spike flips would start to matter).  No on-chip
    transposes or dtype conversions at all.
  - "float32": exact-precision fallback; x is transposed on-chip with
    TensorE and matmuls run in (4-cycle-per-row) fp32 mode.

DMA instruction count is kept low (~75 per core) because each dma_start
occupies the hardware descriptor generator for ~0.6 us: inputs load as a
handful of multi-level-AP DMAs, and the per-timestep repack of Z into the
scan layout [128 q=(b*8+j), 256 f] is a single shape-changing DMA (legal:
dma_start only requires equal element counts; elements pair up in
iteration order, which makes the packed layout a plain row-major reshape).
Spikes go out packed ([T*128, 256]) and are unpacked on the host by
reshape only.
"""

import numpy as np

T, B, IN_SZ, OUT_SZ = 32, 128, 2048, 2048
NCORES = 8
BL = B // NCORES          # 16 batch rows per core
M = T * BL                # 512 matmul rows per core
P = 128
KT = IN_SZ // P           # 16 contraction tiles
NPAN = OUT_SZ // 512      # 4 output panels of 512
MT = M // P               # 4 m-tiles (each = 8 timesteps x 16 batch)
TL = P // BL              # 8 timesteps per m-tile
NJ = OUT_SZ // 256        # 8 column blocks in the packed scan layout
DECAY = 0.8807970779778823  # sigmoid(2.0)
V_TH = 1.0

MM_DTYPE = "fp16fp8"

# fp16fp8 mode: batch x 4 groups (BL2=32/core), out-cols x 2 halves
BL2 = B // 4               # 32 batch rows per core
M2 = T * BL2               # 1024 matmul rows per core
NPAN2 = 2                  # 512-col panels within this core's 1024 cols
MT2 = M2 // P              # 8 m-tiles (each = 4 timesteps x 32 batch)
TL2 = P // BL2             # 4 timesteps per m-tile
CROSS_SCALE = 2.0 ** -19   # fp8 cross passes carry x*2^15(or 2^4) scales

_NC_CACHE = {}


def _build(mm_dtype_name, reps=1, loop_reps=1, ablate=()):
    if mm_dtype_name == "bf16x3":
        return _build_bf16x3(reps, loop_reps, ablate)
    if mm_dtype_name == "fp16fp8":
        return _build_fp16fp8(reps, loop_reps, ablate)
    return _build_f32(mm_dtype_name, reps, loop_reps)


def _scan_tail2(nc, mybir, spool, zs, mt, u, out_d):
    """Scan over the TL2=4 timesteps of one m-tile (fp16fp8 layout).

    zs: [128 m-rows = 4t x 32b, 1024 n] fp32 SBUF.  Two timesteps repack per
    DMA ([64, 1024] -> [128 q=(b*4+j), (t c)]) to halve descriptor-generation
    work, and the repacks alternate between the SP and Act HWDGE queues (both
    idle in the scan phase) so they never serialize on one queue."""
    Alu = mybir.AluOpType
    bf16 = mybir.dt.bfloat16
    f32 = mybir.dt.float32
    st = spool.tile([P, TL2 * 256], bf16, tag=f"st_{mt % 2}", name=f"st_{mt}",
                    bufs=1)
    for tl in range(TL2):
        # m-rows are b-major (m_local = b*4 + tl), so timestep tl's rows are
        # the stride-4 partition slice: [32, 1024] -> [128 q=(b*4+j), 256]
        # is a flat-order reshape with a single 4KB segment per source row.
        zt = spool.tile([P, 256], f32, tag="zt", name="zt", bufs=6)
        eng = nc.sync if (mt * TL2 + tl) % 2 == 0 else nc.scalar
        eng.dma_start(zt[:], zs[tl :: TL2, :])
        ssl = slice(tl * 256, (tl + 1) * 256)
        nc.vector.scalar_tensor_tensor(
            out=u[:], in0=u[:], scalar=DECAY, in1=zt[:],
            op0=Alu.mult, op1=Alu.add,
        )
        nc.vector.tensor_scalar(
            out=st[:, ssl], in0=u[:], scalar1=V_TH, scalar2=None, op0=Alu.is_ge
        )
        nc.vector.tensor_tensor(
            out=u[:], in0=u[:], in1=st[:, ssl], op=Alu.subtract
        )
    out_v = out_d[:].rearrange("(t q) f -> q t f", q=P)
    nc.gpsimd.dma_start(out_v[:, mt * TL2 : (mt + 1) * TL2, :], st[:])


def _build_fp16fp8(reps=1, loop_reps=1, ablate=()):
    """fp16 main pass (1 cyc/row) + two fp8-e4m3 DoubleRow cross passes
    (0.5 cyc/row each) = 2.0 bf16-pass equivalents of PE time.

    z = fp16(x)@fp16(w) + 2^-19*( e4m3(xl*2^15)@e4m3(w*2^4)
                                + e4m3(x*2^4)@e4m3(wl*2^15) )
    where xl = x - fp16(x), wl = w - fp16(w).  z-rmse ~1e-5 -> ~40 spike
    flips out of 1.09M spikes (gate allows 435).

    Sharding: batch 4-way x out-cols 2-way (core c: batch g=c//2, half
    h=c%2).  Per core the matmul is [1024, 2048] @ [2048, 1024]."""
    from contextlib import ExitStack

    from concourse import bacc, mybir
    from concourse.tile import TileContext

    f32 = mybir.dt.float32
    f16 = mybir.dt.float16
    fp8 = mybir.dt.float8e4
    DR = mybir.MatmulPerfMode.DoubleRow

    nc = bacc.Bacc()
    xh_d = nc.declare_dram_parameter("xh", [IN_SZ, M2], f16, isOutput=False)
    xl8_d = nc.declare_dram_parameter("xl8", [IN_SZ, M2], fp8, isOutput=False)
    wh_d = nc.declare_dram_parameter("wh", [IN_SZ, 1024], f16, isOutput=False)
    w8_d = nc.declare_dram_parameter("w8", [IN_SZ, 1024], fp8, isOutput=False)
    wl8_d = nc.declare_dram_parameter("wl8", [IN_SZ, 1024], fp8, isOutput=False)
    out_d = nc.declare_dram_parameter("out", [T * P, 256], mybir.dt.bfloat16,
                                      isOutput=True)

    xh_v = xh_d[:].rearrange("(kt p) m -> p kt m", p=P)
    xl8_v = xl8_d[:].rearrange("(kt p) m -> p kt m", p=P)
    wh_v = wh_d[:].rearrange("(kt p) n -> p kt n", p=P)
    w8_v = w8_d[:].rearrange("(kt p) n -> p kt n", p=P)
    wl8_v = wl8_d[:].rearrange("(kt p) n -> p kt n", p=P)

    with ExitStack() as ctx:
        tc = ctx.enter_context(TileContext(nc))
        wpool = ctx.enter_context(tc.tile_pool(name="wpool", bufs=1))
        xtpool = ctx.enter_context(tc.tile_pool(name="xtpool", bufs=1))
        zpool = ctx.enter_context(tc.tile_pool(name="zpool", bufs=1))
        spool = ctx.enter_context(tc.tile_pool(name="spool", bufs=3))
        upool = ctx.enter_context(tc.tile_pool(name="upool", bufs=1))
        psum = ctx.enter_context(tc.tile_pool(name="psum", bufs=4, space="PSUM"))

        def _rep_body():
            xh = xtpool.tile([P, KT, M2], f16, tag="xh", name="xh")
            xl8 = xtpool.tile([P, KT, M2], fp8, tag="xl8", name="xl8")
            x8 = xtpool.tile([P, KT, M2], fp8, tag="x8", name="x8")
            wh = [wpool.tile([P, KT, 512], f16, tag=f"wh_{n}", name=f"wh_{n}")
                  for n in range(NPAN2)]
            w8 = [wpool.tile([P, KT, 512], fp8, tag=f"w8_{n}", name=f"w8_{n}")
                  for n in range(NPAN2)]
            wl8 = [wpool.tile([P, KT, 512], fp8, tag=f"wl8_{n}", name=f"wl8_{n}")
                   for n in range(NPAN2)]

            def _x_chunk(c, lo=0, hi=256):
                msl = slice(c * 256 + lo, c * 256 + hi)
                nc.sync.dma_start(xh[:, :, msl], xh_v[:, :, msl])

            def _xl8_half(k):
                ksl = slice(k * (KT // 2), (k + 1) * (KT // 2))
                nc.sync.dma_start(xl8[:, ksl, :], xl8_v[:, ksl, :])

            def _wh_chunk(np_, q, nq=4):
                hk = KT // nq
                ksl = slice(q * hk, (q + 1) * hk)
                nsl = slice(np_ * 512, (np_ + 1) * 512)
                nc.sync.dma_start(wh[np_][:, ksl, :], wh_v[:, ksl, nsl])

            # DMA stream (sync queue), ordered by first use.  x8 is NOT
            # shipped: it is derived on-chip from xh by Act scaled casts
            # (xh lands early, so the derives are never on the critical
            # path), cutting input DMA from 16 to 14 MB per core.  The first
            # m-block of x goes alone so the first matmul chain starts early.
            _x_chunk(0, 0, 128)
            _wh_chunk(0, 0)
            _x_chunk(0, 128, 256)
            for q in range(1, 4):
                _wh_chunk(0, q)
            _x_chunk(1)
            _xl8_half(0)
            _xl8_half(1)
            nsl0 = slice(0, 512)
            nc.sync.dma_start(w8[0][:], w8_v[:, :, nsl0])
            nc.sync.dma_start(wl8[0][:], wl8_v[:, :, nsl0])
            _x_chunk(2)
            _x_chunk(3)
            for q in range(4):
                _wh_chunk(1, q)
            nsl1 = slice(512, 1024)
            nc.sync.dma_start(w8[1][:], w8_v[:, :, nsl1])
            nc.sync.dma_start(wl8[1][:], wl8_v[:, :, nsl1])

            def _derive_x8(c):
                msl = slice(c * 256, (c + 1) * 256)
                nc.scalar.mul(x8[:, :, msl], xh[:, :, msl], 2.0**4)

            u = upool.tile([P, 256], f32, name="u")
            nc.vector.memset(u[:], 0.0)

            zss = [zpool.tile([P, 1024], f32, tag=f"zs_{mt}", name=f"zs_{mt}")
                   for mt in range(MT2)]

            # Software pipeline: main fp16 chain for group i issues together
            # with the fp8 cross chain + merge (+scan) of group i-LAG, so the
            # in-order PE never waits on the (later-arriving) cross operands.
            groups = [(np_, mt) for np_ in range(NPAN2) for mt in range(MT2)]
            LAG = 4
            # Act-engine derives interleaved at the right program points
            # (Act executes in order; derives must not block PSUM copies)
            derives = {0: [lambda: _derive_x8(0)],
                       1: [lambda: _derive_x8(1)],
                       3: [lambda: _derive_x8(2)],
                       5: [lambda: _derive_x8(3)]}
            zpa = {}
            zpb = {}

            def emit_main(i):
                np_, mt = groups[i]
                msl = slice(mt * P, (mt + 1) * P)
                zpa[i] = psum.tile([P, 512], f32, tag="psA", name="zpa", bufs=6)
                if "mm" not in ablate:
                    for kt in range(KT):
                        nc.tensor.matmul(
                            zpa[i][:], xh[:, kt, msl], wh[np_][:, kt, :],
                            start=(kt == 0), stop=(kt == KT - 1),
                        )
                else:
                    nc.tensor.matmul(zpa[i][:], xh[:, 0, msl], wh[np_][:, 0, :],
                                     start=True, stop=True)

            def emit_tail(j):
                np_, mt = groups[j]
                msl = slice(mt * P, (mt + 1) * P)
                nsl = slice(np_ * 512, (np_ + 1) * 512)
                if "cross" not in ablate:
                    zpb[j] = psum.tile([P, 512], f32, tag="psB", name="zpb",
                                       bufs=2)
                    for kt in range(0, KT, 2):
                        ksl = slice(kt, kt + 2)
                        nc.tensor.matmul(
                            zpb[j][:], xl8[:, ksl, msl], w8[np_][:, ksl, :],
                            perf_mode=DR, start=(kt == 0), stop=False,
                        )
                        nc.tensor.matmul(
                            zpb[j][:], x8[:, ksl, msl], wl8[np_][:, ksl, :],
                            perf_mode=DR, start=False, stop=(kt == KT - 2),
                        )
                nc.scalar.copy(out=zss[mt][:, nsl], in_=zpa[j][:])
                if "cross" not in ablate:
                    # (GPSIMD cannot read PSUM, so the merge stays on DVE)
                    nc.vector.scalar_tensor_tensor(
                        out=zss[mt][:, nsl], in0=zpb[j][:],
                        scalar=CROSS_SCALE, in1=zss[mt][:, nsl],
                        op0=mybir.AluOpType.mult, op1=mybir.AluOpType.add,
                    )
                if np_ == NPAN2 - 1 and "scan" not in ablate:
                    _scan_tail2(nc, mybir, spool, zss[mt], mt, u, out_d)

            # np0 (groups 0..7): crosses trail mains by LAG so the in-order
            # PE is never blocked on late-arriving fp8/weight DMAs during
            # ramp-up.  np1 (groups 8..15): everything is resident, and each
            # m-tile's scan is gated on its np1 cross — so run main, cross,
            # merge, scan back-to-back per group to feed the serial scan
            # chain as early as possible.
            half = len(groups) // 2
            for i in range(half):
                for d in derives.get(i, ()):
                    d()
                emit_main(i)
                if i - LAG >= 0:
                    emit_tail(i - LAG)
            for j in range(half - LAG, half):
                emit_tail(j)
            for i in range(half, len(groups)):
                for d in derives.get(i, ()):
                    d()
                emit_main(i)
                emit_tail(i)

        if loop_reps > 1:
            with tc.For_i(0, loop_reps, 1):
                _rep_body()
        else:
            for _ in range(reps):
                _rep_body()

    nc.finalize()
    return nc


def _scan_tail(nc, mybir, spool, zs, mt, u, out_d):
    """Nonlinear scan over the 8 timesteps of one m-tile + packed output.

    zs: [128 m-rows, 2048 n] fp32 in SBUF.  For each timestep the 16 rows
    (t fixed, b varying) are repacked into [128 q=(b*8+j), 256] with one
    shape-changing DMA so all 128 DVE lanes work.  Spikes accumulate in a
    per-mt staging tile and leave in a single DMA; scan-path DMAs ride the
    otherwise-idle GpSimd SWDGE queue so they never contend with the
    HWDGE input stream.
    """
    Alu = mybir.AluOpType
    bf16 = mybir.dt.bfloat16
    f32 = mybir.dt.float32
    # spikes are exactly 0.0/1.0 -> bf16 staging is lossless and halves
    # both SBUF footprint and output-DMA bytes (host upcasts to f32)
    st = spool.tile([P, TL * 256], bf16, tag=f"st_{mt % 2}", name=f"st_{mt}",
                    bufs=1)
    for tl in range(TL):
        zt = spool.tile([P, 256], f32, tag="zt", name="zt", bufs=6)
        nc.gpsimd.dma_start(zt[:], zs[tl * BL : (tl + 1) * BL, :])
        # u = u*decay + z_t ; s = (u >= vth) ; u -= s
        nc.vector.scalar_tensor_tensor(
            out=u[:], in0=u[:], scalar=DECAY, in1=zt[:],
            op0=Alu.mult, op1=Alu.add,
        )
        ssl = slice(tl * 256, (tl + 1) * 256)
        nc.vector.tensor_scalar(
            out=st[:, ssl], in0=u[:], scalar1=V_TH, scalar2=None, op0=Alu.is_ge
        )
        nc.vector.tensor_tensor(
            out=u[:], in0=u[:], in1=st[:, ssl], op=Alu.subtract
        )
    # one DMA for the whole m-tile: [128 q, 8t x 256] -> rows (t*128+q)
    out_v = out_d[:].rearrange("(t q) f -> q t f", q=P)
    nc.gpsimd.dma_start(out_v[:, mt * TL : (mt + 1) * TL, :], st[:])


def _build_bf16x3(reps=1, loop_reps=1, ablate=()):
    from contextlib import ExitStack

    from concourse import bacc, mybir
    from concourse.tile import TileContext

    f32 = mybir.dt.float32
    bf16 = mybir.dt.bfloat16

    nc = bacc.Bacc()
    # Host-prepared: x transposed+split, W split (all bf16, natural layout).
    xht_d = nc.declare_dram_parameter("xht", [IN_SZ, M], bf16, isOutput=False)
    xlt_d = nc.declare_dram_parameter("xlt", [IN_SZ, M], bf16, isOutput=False)
    wh_d = nc.declare_dram_parameter("wh", [IN_SZ, OUT_SZ], bf16, isOutput=False)
    wl_d = nc.declare_dram_parameter("wl", [IN_SZ, OUT_SZ], bf16, isOutput=False)
    out_d = nc.declare_dram_parameter("out", [T * P, 256], bf16, isOutput=True)

    # [IN_SZ, ...] DRAM views iterated (p, kt, cols): one DMA fills a whole
    # [128, KT*cols] SBUF tile.
    xht_v = xht_d[:].rearrange("(kt p) m -> p kt m", p=P)
    xlt_v = xlt_d[:].rearrange("(kt p) m -> p kt m", p=P)
    wh_v = wh_d[:].rearrange("(kt p) n -> p kt n", p=P)
    wl_v = wl_d[:].rearrange("(kt p) n -> p kt n", p=P)

    with ExitStack() as ctx:
        tc = ctx.enter_context(TileContext(nc))
        wpool = ctx.enter_context(tc.tile_pool(name="wpool", bufs=1))
        xtpool = ctx.enter_context(tc.tile_pool(name="xtpool", bufs=1))
        zpool = ctx.enter_context(tc.tile_pool(name="zpool", bufs=1))
        spool = ctx.enter_context(tc.tile_pool(name="spool", bufs=3))
        upool = ctx.enter_context(tc.tile_pool(name="upool", bufs=1))
        psum = ctx.enter_context(tc.tile_pool(name="psum", bufs=8, space="PSUM"))

        def _rep_body():
            # x tiles: [128, KT*M] = all k-tiles side by side.
            # W tiles: one [128, KT*512] tile per (hi/lo, panel).
            # DMAs are emitted in first-use order, sliced finest at the
            # front (2-kt x slices, quarter W panels) so the first matmul
            # group waits on ~2 MiB instead of the whole input set.
            xh = xtpool.tile([P, KT * M], bf16, tag="xh", name="xh")
            xl = xtpool.tile([P, KT * M], bf16, tag="xl", name="xl")
            wh = [
                wpool.tile([P, KT * 512], bf16, tag=f"wh_{np_}", name=f"wh_{np_}")
                for np_ in range(NPAN)
            ]
            wl = [
                wpool.tile([P, KT * 512], bf16, tag=f"wl_{np_}", name=f"wl_{np_}")
                for np_ in range(NPAN)
            ]

            def _x_chunk(q):
                ksl = slice(q * 4, (q + 1) * 4)
                fsl = slice(q * 4 * M, (q + 1) * 4 * M)
                nc.sync.dma_start(xh[:, fsl], xht_v[:, ksl, :])
                nc.sync.dma_start(xl[:, fsl], xlt_v[:, ksl, :])

            def _w_chunk(np_, h, nh=2):
                hk = KT // nh
                ksl = slice(h * hk, (h + 1) * hk)
                fsl = slice(h * hk * 512, (h + 1) * hk * 512)
                nsl = slice(np_ * 512, (np_ + 1) * 512)
                nc.sync.dma_start(wh[np_][:, fsl], wh_v[:, ksl, nsl])
                nc.sync.dma_start(wl[np_][:, fsl], wl_v[:, ksl, nsl])

            def _x_fine(e):
                # 2-kt slices for the very front of the stream
                ksl = slice(e * 2, (e + 1) * 2)
                fsl = slice(e * 2 * M, (e + 1) * 2 * M)
                nc.sync.dma_start(xh[:, fsl], xht_v[:, ksl, :])
                nc.sync.dma_start(xl[:, fsl], xlt_v[:, ksl, :])

            _x_fine(0)
            _w_chunk(0, 0, nh=4)
            _x_fine(1)
            _w_chunk(0, 1, nh=4)
            _x_chunk(1)
            _w_chunk(0, 1, nh=2)
            _x_chunk(2)
            _x_chunk(3)
            for np_ in range(1, NPAN):
                _w_chunk(np_, 0, nh=1)

            u = upool.tile([P, 256], f32, name="u")
            nc.vector.memset(u[:], 0.0)

            # np-outer so each W panel is fully consumed (all 4 m-tiles)
            # right as it streams in; Z rows accumulate in per-mt SBUF
            # buffers and each scan fires once its row is complete.
            zss = [
                zpool.tile([P, OUT_SZ], f32, tag=f"zs_{mt}", name=f"zs_{mt}")
                for mt in range(MT)
            ]
            for np_ in range(NPAN):
                for mt in range(MT):
                    zp = psum.tile([P, 512], f32, tag="ps", name="zp")
                    if "mm" not in ablate:
                        for kt in range(KT):
                            xsl = slice(kt * M + mt * P, kt * M + (mt + 1) * P)
                            wsl = slice(kt * 512, (kt + 1) * 512)
                            nc.tensor.matmul(
                                zp[:], xh[:, xsl], wh[np_][:, wsl],
                                start=(kt == 0), stop=False,
                            )
                            nc.tensor.matmul(
                                zp[:], xh[:, xsl], wl[np_][:, wsl],
                                start=False, stop=False,
                            )
                            nc.tensor.matmul(
                                zp[:], xl[:, xsl], wh[np_][:, wsl],
                                start=False, stop=(kt == KT - 1),
                            )
                    else:
                        nc.tensor.matmul(
                            zp[:], xh[:, mt * P : (mt + 1) * P],
                            wh[np_][:, :512], start=True, stop=True,
                        )
                    nc.scalar.copy(
                        out=zss[mt][:, np_ * 512 : (np_ + 1) * 512], in_=zp[:]
                    )
                    if np_ == NPAN - 1 and "scan" not in ablate:
                        _scan_tail(nc, mybir, spool, zss[mt], mt, u, out_d)

        if loop_reps > 1:
            with tc.For_i(0, loop_reps, 1):
                _rep_body()
        else:
            for _ in range(reps):
                _rep_body()

    nc.finalize()
    return nc


def _build_f32(mm_dtype_name, reps=1, loop_reps=1):
    from contextlib import ExitStack

    from concourse import bacc, mybir
    from concourse.tile import TileContext

    f32 = mybir.dt.float32
    dt_mm = getattr(mybir.dt, mm_dtype_name)

    nc = bacc.Bacc()
    x_d = nc.declare_dram_parameter("x", [M, IN_SZ], f32, isOutput=False)
    w_d = nc.declare_dram_parameter("w", [IN_SZ, OUT_SZ], f32, isOutput=False)
    out_d = nc.declare_dram_parameter(
        "out", [T * P, 256], mybir.dt.bfloat16, isOutput=True
    )
    w_v = w_d[:].rearrange("(kt p) n -> p kt n", p=P)

    with ExitStack() as ctx:
        tc = ctx.enter_context(TileContext(nc))
        const_pool = ctx.enter_context(tc.tile_pool(name="const", bufs=1))
        wpool = ctx.enter_context(tc.tile_pool(name="wpool", bufs=1))
        xtpool = ctx.enter_context(tc.tile_pool(name="xtpool", bufs=1))
        xstage = ctx.enter_context(tc.tile_pool(name="xstage", bufs=1))
        spool = ctx.enter_context(tc.tile_pool(name="spool", bufs=3))
        upool = ctx.enter_context(tc.tile_pool(name="upool", bufs=1))
        psum = ctx.enter_context(tc.tile_pool(name="psum", bufs=8, space="PSUM"))

        ident = const_pool.tile([P, P], f32, name="ident")
        nc.gpsimd.memset(ident[:], 0.0)
        nc.gpsimd.affine_select(
            out=ident[:], in_=ident[:],
            compare_op=mybir.AluOpType.not_equal,
            fill=1.0, base=0, pattern=[[-1, P]], channel_multiplier=1,
        )

        def _rep_body():
            # x first (the transposes gate the whole pipeline)
            xt = [
                xtpool.tile([P, M], f32, tag=f"xt_{kt}", name=f"xt_{kt}")
                for kt in range(KT)
            ]
            xs = [
                xstage.tile([P, IN_SZ], f32, tag=f"xs_{mt}", name=f"xs_{mt}")
                for mt in range(MT)
            ]
            for mt in range(MT):
                nc.sync.dma_start(xs[mt][:], x_d[mt * P : (mt + 1) * P, :])

            # W: one [128, KT*512] tile per panel, 1 DMA each
            wt = [
                wpool.tile([P, KT * 512], f32, tag=f"w_{np_}", name=f"w_{np_}")
                for np_ in range(NPAN)
            ]
            for np_ in range(NPAN):
                nc.sync.dma_start(
                    wt[np_][:], w_v[:, :, np_ * 512 : (np_ + 1) * 512]
                )

            # TensorE transposes: kt-outer/mt-inner, 4 per PSUM tile
            for kt in range(KT):
                pt = psum.tile([P, 512], f32, tag="ps", name="pt")
                for mt in range(MT):
                    nc.tensor.transpose(
                        pt[:, mt * P : (mt + 1) * P],
                        xs[mt][:, kt * P : (kt + 1) * P],
                        ident[:],
                    )
                nc.any.tensor_copy(out=xt[kt][:], in_=pt[:])

            u = upool.tile([P, 256], f32, name="u")
            nc.vector.memset(u[:], 0.0)

            for mt in range(MT):
                zps = []
                for np_ in range(NPAN):
                    zp = psum.tile([P, 512], f32, tag="ps", name="zp")
                    for kt in range(KT):
                        nc.tensor.matmul(
                            zp[:],
                            xt[kt][:, mt * P : (mt + 1) * P].bitcast(dt_mm),
                            wt[np_][:, kt * 512 : (kt + 1) * 512].bitcast(dt_mm),
                            start=(kt == 0),
                            stop=(kt == KT - 1),
                        )
                    zps.append(zp)
                # reuse the x staging slots for the Z panels (x is dead)
                zs = xstage.tile([P, OUT_SZ], f32, tag=f"xs_{mt}",
                                 name=f"zs_{mt}")
                for np_ in range(NPAN):
                    nc.any.tensor_copy(
                        out=zs[:, np_ * 512 : (np_ + 1) * 512], in_=zps[np_][:]
                    )
                _scan_tail(nc, mybir, spool, zs, mt, u, out_d)

        if loop_reps > 1:
            with tc.For_i(0, loop_reps, 1):
                _rep_body()
        else:
            for _ in range(reps):
                _rep_body()

    nc.finalize()
    return nc


def _get_nc():
    key = MM_DTYPE
    if key not in _NC_CACHE:
        _NC_CACHE[key] = _build(key)
    return _NC_CACHE[key]


def _split_bf16(a):
    import ml_dtypes

    hi = a.astype(ml_dtypes.bfloat16)
    lo = (a - hi.astype(np.float32)).astype(ml_dtypes.bfloat16)
    return hi, lo


def make_in_maps(x, w):
    # Shard + host-side layout prep (transpose/split) for the current mode.
    x = np.ascontiguousarray(x, dtype=np.float32)
    w = np.ascontiguousarray(w, dtype=np.float32)
    in_maps = []
    if MM_DTYPE == "fp16fp8":
        import ml_dtypes

        e4 = ml_dtypes.float8_e4m3
        wmaps = []
        for h in range(2):
            wc = np.ascontiguousarray(w[:, h * 1024 : (h + 1) * 1024])
            wh = wc.astype(np.float16)
            wl = wc - wh.astype(np.float32)
            wmaps.append({
                "wh": wh,
                "w8": (wc * 2.0**4).astype(e4),
                "wl8": (wl * 2.0**15).astype(e4),
            })
        xmaps = []
        for g in range(4):
            xs = x[:, g * BL2 : (g + 1) * BL2, :]
            # b-major m-order inside each 128-row m-tile (m = b*TL2 + tl) so
            # the scan's per-timestep repack is a stride-4 partition slice
            xs = xs.reshape(MT2, TL2, BL2, IN_SZ).transpose(0, 2, 1, 3)
            xt = np.ascontiguousarray(xs.reshape(M2, IN_SZ).T)
            xh = xt.astype(np.float16)
            xl = xt - xh.astype(np.float32)
            xmaps.append({
                "xh": xh,
                "xl8": (xl * 2.0**15).astype(e4),
            })
        for c in range(NCORES):
            in_maps.append({**xmaps[c // 2], **wmaps[c % 2]})
        return in_maps
    if MM_DTYPE == "bf16x3":
        wh, wl = _split_bf16(w)
        for c in range(NCORES):
            xs = x[:, c * BL : (c + 1) * BL, :].reshape(M, IN_SZ)
            xst = np.ascontiguousarray(xs.T)          # [IN_SZ, M] fp32
            xht, xlt = _split_bf16(xst)
            in_maps.append(
                {"xht": np.ascontiguousarray(xht),
                 "xlt": np.ascontiguousarray(xlt),
                 "wh": wh, "wl": wl}
            )
    else:
        for c in range(NCORES):
            xs = np.ascontiguousarray(
                x[:, c * BL : (c + 1) * BL, :].reshape(M, IN_SZ)
            )
            in_maps.append({"x": xs, "w": w})
    return in_maps


def kernel(x, kernel):
    from concourse.bass_utils import run_bass_kernel_spmd

    nc = _get_nc()
    in_maps = make_in_maps(x, kernel)
    res = run_bass_kernel_spmd(nc, in_maps, core_ids=list(range(NCORES)))

    if MM_DTYPE == "fp16fp8":
        # per core: row = t*128 + b*4 + j, j over four 256-col blocks of the
        # core's 1024-col half -> reshape to [T, 32, 1024]
        full = np.empty((T, B, OUT_SZ), np.float32)
        for c in range(NCORES):
            g, h = c // 2, c % 2
            o = res.results[c]["out"].astype(np.float32)
            o = o.reshape(T, BL2, 1024)
            full[:, g * BL2 : (g + 1) * BL2, h * 1024 : (h + 1) * 1024] = o
        return full

    outs = []
    for c in range(NCORES):
        # [T*128, 256] packed, row = t*128 + b*8 + j  ->  plain reshape
        # (bf16 spike values are exactly 0.0/1.0; upcast restores f32)
        o = res.results[c]["out"].astype(np.float32).reshape(T, BL, OUT_SZ)
        outs.append(o)
    return np.concatenate(outs, axis=1)

